# revision 23
# baseline (speedup 1.0000x reference)
"""AdptWeightBCEDiceLoss on 8 TRN2 NeuronCores — pure data parallel.

Full inputs y_pred/y_target [32,1,512,512] f32 are cast to bf16 on host
and sharded 4 images per core.  All elementwise work runs in TRANSPOSED
image space (host also ships x^T and t^T); only box-filter stage 1
consumes t in normal orientation.  Each core emits per-image partial
sums; the host combines 8x12x4 partials into the final scalar (the
"all-reduce" of the sharding hint).

Per image (N = 512*512, e = 1e-8, s = 5/961):
  p     = sigmoid(xT)                      ScalarE, accum -> Sp
  Y     = B_ones @ t   (col box sum)       TensorE -> PSUM, evac bf16
  btps  = B_ones @ Y^T - 961 tT            TensorE (fold into one PSUM)
  W5T   = s|btps|                          ScalarE Abs, accum -> Sw5
  c2T   = pT+tT          (accum Sc2)       DVE STT
  ptT   = pT*tT                            DVE TT (2x mode)
  inter = sum(ptT*(1+W5T))                 DVE STT (scalar port adds 1)
  union = sum(c2T*(1+W5T))                 DVE STT
  Stx   = sum(tT*xT)                       DVE STT
  q     = (pT-mp)^2 + (tT-mt)^2            DVE custom
  v     = (q+e)^-1/2                       ScalarE Abs_reciprocal_sqrt
  Sg2   = sum((((c2T-S0)^2+2e) * v^2)^2)   DVE custom w/ accum (= sum G^2)
  Slnp  = sum(ln pT) (global)              ScalarE Ln
"""

import numpy as np
import ml_dtypes

import concourse.bass as bass
import concourse.mybir as mybir
import concourse.tile as tile
import concourse.bacc as bacc
from concourse.bass_utils import run_bass_kernel_spmd

# ---------------------------------------------------------------- constants
N_CORES = 8
IMG = 4           # images per core
HB = 4            # 128-row blocks per image
W = 512
P = 128
NPIX = 512 * 512
EPS = 1e-8
SCOLS = 12        # stats columns per image
F32 = mybir.dt.float32
BF16 = mybir.dt.bfloat16
FP8 = mybir.dt.float8e4
AX = mybir.AluOpType

# stats columns (per image)
C_SP, C_SC2, C_SW5, C_INT, C_UNI, C_SG2, C_STX, C_SLNP = range(8)

# ------------------------------------------------------- custom DVE ops
from concourse import dve_ops as _dvo
from concourse.dve_spec import Spec, Src0, Src1, C0, C1, C2, sq, lower, _has_src1, AluOp
from concourse.dve_uop import DveOpSpec


def _register(name, spec, subdim=False):
    for op in _dvo.OPS:
        if op.name == name:
            return op
    row = _dvo._CUSTOM_DVE_ROW_BASE + len(_dvo.OPS)
    assert row < 0x20
    shas = {}
    for ver in ("v3",):
        tmp = DveOpSpec(name=name, opcode=row, uops=lower(spec, ver=ver),
                        rd1_en=_has_src1(spec))
        shas[ver] = tmp.sha(ver)
    op = _dvo.DveOp(name, spec, subdim, shas)
    _dvo.OPS.append(op)
    _dvo._SUB_OPCODE_FOR_NAME[name] = row
    _dvo.CUSTOM_DVE_SPECS[name] = spec
    return op


# q = (p - mp)^2 + (t - mt)^2
ATH_Q = _register(
    "ATH_Q2",
    Spec(
        body=sq(Src0 - C0) + sq(Src1 - C1),
        reference=lambda in0, in1, s0, s1, imm2: (
            (in0.astype(np.float32) - s0) ** 2
            + (in1.astype(np.float32) - s1) ** 2
        ).astype(np.float32),
    ),
)

# G^2 = (((c2 - S0)^2 + 2e) * v^2)^2 with v = rsqrt(q+e); accum -> sum G^2
ATH_QFM = _register(
    "ATH_QFM",
    Spec(
        body=sq((sq(Src0 - C0) + C1) * sq(Src1)),
        accum=AluOp.ADD,
        reference=lambda in0, in1, s0, s1, imm2: (
            (((in0.astype(np.float32) - s0) ** 2 + s1)
             * in1.astype(np.float32) ** 2) ** 2
        ).astype(np.float32),
    ),
)


def band_consts():
    idx = np.arange(256)
    B = (np.abs(idx[:, None] - idx[None, :]) <= 15).astype(np.float32)
    D = B[0:128, 0:128]      # same-block band (ones)
    UP = B[0:128, 128:256]   # input block j = out block - 1
    DN = B[128:256, 0:128]   # input block j = out block + 1
    I = np.eye(128, dtype=np.float32)
    bf = ml_dtypes.bfloat16
    f8 = ml_dtypes.float8_e4m3fn
    return {
        "bandD": D.astype(f8), "bandUp": UP.astype(f8), "bandDn": DN.astype(f8),
        "bandDb": D.astype(bf), "bandUpb": UP.astype(bf), "bandDnb": DN.astype(bf),
        "negI961": (-961.0 * I).astype(bf),
    }


def _fl(ap):
    if len(ap.shape) == 3:
        return ap.rearrange("p a b -> p (a b)")
    if len(ap.shape) == 4:
        return ap.rearrange("p a b c -> p (a b c)")
    return ap


# ------------------------------------------------------------- builder
import os
SKIP = set(os.environ.get("ATH_SKIP", "").split(","))


def build_nc():
    nc = bacc.Bacc("TRN2", target_bir_lowering=False, debug=False,
                   num_devices=N_CORES)
    # xT/tT: transposed images [i, w, h]; tn: normal [i, h, w]
    xTd = nc.dram_tensor("xT", [IMG, 512, 512], FP8, kind="ExternalInput").ap()
    tTd = nc.dram_tensor("tT", [IMG, 512, 512], BF16, kind="ExternalInput").ap()
    tnd = nc.dram_tensor("tn", [IMG, 512, 512], FP8, kind="ExternalInput").ap()
    cD = nc.dram_tensor("bandD", [P, P], FP8, kind="ExternalInput").ap()
    cUp = nc.dram_tensor("bandUp", [P, P], FP8, kind="ExternalInput").ap()
    cDn = nc.dram_tensor("bandDn", [P, P], FP8, kind="ExternalInput").ap()
    cNI = nc.dram_tensor("negI961", [P, P], BF16, kind="ExternalInput").ap()
    cDb = nc.dram_tensor("bandDb", [P, P], BF16, kind="ExternalInput").ap()
    cUpb = nc.dram_tensor("bandUpb", [P, P], BF16, kind="ExternalInput").ap()
    cDnb = nc.dram_tensor("bandDnb", [P, P], BF16, kind="ExternalInput").ap()
    outv = nc.dram_tensor("out", [1, SCOLS * IMG], F32, kind="ExternalOutput").ap()

    xTv = xTd.rearrange("i (b p) h -> p (i b) h", p=P)
    tTv = tTd.rearrange("i (b p) h -> p (i b) h", p=P)
    tnv = tnd.rearrange("i (b p) w -> p (i b) w", p=P)

    with tile.TileContext(nc) as tc:
        import contextlib
        ctx = contextlib.ExitStack()
        with ctx:
            cpool = ctx.enter_context(tc.tile_pool(name="consts", bufs=1))
            dpool = ctx.enter_context(tc.tile_pool(name="data", bufs=1))
            ppool = ctx.enter_context(tc.tile_pool(name="ptc2", bufs=1))
            wpool = ctx.enter_context(tc.tile_pool(name="w5", bufs=3))
            ypool = ctx.enter_context(tc.tile_pool(name="ybuf", bufs=2))
            jpool = ctx.enter_context(tc.tile_pool(name="junk", bufs=4))
            qpool = ctx.enter_context(tc.tile_pool(name="qv", bufs=2))
            mpool = ctx.enter_context(tc.tile_pool(name="means", bufs=4))
            pspool = ctx.enter_context(tc.tile_pool(name="ps", bufs=2, space="PSUM"))

            bD = cpool.tile([P, P], FP8, tag="bD")
            bUp = cpool.tile([P, P], FP8, tag="bUp")
            bDn = cpool.tile([P, P], FP8, tag="bDn")
            nI = cpool.tile([P, P], BF16, tag="nI")
            bDb = cpool.tile([P, P], BF16, tag="bDb")
            bUpb = cpool.tile([P, P], BF16, tag="bUpb")
            bDnb = cpool.tile([P, P], BF16, tag="bDnb")
            ebias = cpool.tile([P, 1], F32, tag="ebias")
            nc.gpsimd.memset(ebias[:], EPS)

            xbf = dpool.tile([P, IMG * HB, W], FP8, tag="xbf")   # xT
            tbf = dpool.tile([P, IMG * HB, W], BF16, tag="tbf")   # tT
            tnb = dpool.tile([P, IMG * HB, W], FP8, tag="tnb")    # t normal
            pbf = dpool.tile([P, IMG * HB, W], BF16, tag="pbf")   # pT
            stats_t = []
            for j in range(IMG):
                st_j = dpool.tile([P, SCOLS], F32, tag=f"stats{j}")
                stats_t.append(st_j)
            arout = dpool.tile([P, SCOLS * IMG], F32, tag="arout")
            pt_t, c2_t, w5_t, msums, s0d0s, vmaps = [], [], [], [], [], []

            # consts via SWDGE (tiny, land early); inputs as per-image trios
            # on the sync queue so image 0 is complete ASAP.
            nc.gpsimd.dma_start(out=bD[:], in_=cD)
            nc.gpsimd.dma_start(out=bUp[:], in_=cUp)
            nc.gpsimd.dma_start(out=bDn[:], in_=cDn)
            nc.gpsimd.dma_start(out=nI[:], in_=cNI)
            nc.gpsimd.dma_start(out=bDb[:], in_=cDb)
            nc.gpsimd.dma_start(out=bUpb[:], in_=cUpb)
            nc.gpsimd.dma_start(out=bDnb[:], in_=cDnb)
            for i in range(IMG):
                sl = slice(HB * i, HB * i + HB)
                nc.sync.dma_start(tbf[:, sl, :], tTv[:, sl, :])
                nc.sync.dma_start(xbf[:, sl, :], xTv[:, sl, :])
                nc.sync.dma_start(tnb[:, sl, :], tnv[:, sl, :])

            import concourse.bass_isa as bass_isa

            # all sigmoids first: Scalar queue never returns to this table
            for i in range(IMG):
                sl = slice(HB * i, HB * i + HB)
                nc.scalar.activation(_fl(pbf[:, sl, :]), _fl(xbf[:, sl, :]),
                                     mybir.ActivationFunctionType.Sigmoid,
                                     accum_out=stats_t[i][:, C_SP:C_SP + 1])

            def emeasure(k):
                """E-measure chain for image k (maps built in iter k)."""
                sl = slice(HB * k, HB * k + HB)
                msum = msums[k]
                s0d0 = mpool.tile([P, 3], F32, tag="s0d0")  # [mp, mt, S0]
                # [mp, S0] in one op, then mt = S0 - mp
                nc.vector.tensor_single_scalar(s0d0[:, 0:2], msum[:, 0:2],
                                               1.0 / NPIX, AX.mult)
                nc.vector.scalar_tensor_tensor(
                    s0d0[:, 2:3], s0d0[:, 0:1], -1.0, s0d0[:, 1:2],
                    op0=AX.mult, op1=AX.add)  # mt = S0 - mp
                q = qpool.tile([P, HB, W], BF16, tag="q")
                nc.vector._custom_dve(ATH_Q, out=_fl(q[:]),
                                      in0=_fl(pbf[:, sl, :]),
                                      in1=_fl(tbf[:, sl, :]),
                                      s0=s0d0[:, 0:1], s1=s0d0[:, 2:3])
                v = qpool.tile([P, HB, W], BF16, tag="v")
                nc.scalar.activation(
                    _fl(v[:]), _fl(q[:]),
                    mybir.ActivationFunctionType.Abs_reciprocal_sqrt,
                    bias=ebias[:])
                s0d0s.append(s0d0)
                vmaps.append(v)

            def scsums(k):
                # inter = sum(pt*(1+W5)), union = sum(c2*(1+W5)); the +1 is
                # folded into the STT scalar port
                j1 = jpool.tile([P, HB, W], BF16, tag="junk")
                j2 = jpool.tile([P, HB, W], BF16, tag="junk")
                nc.vector.scalar_tensor_tensor(
                    _fl(j1[:]), _fl(w5_t[k][:]), 1.0, _fl(pt_t[k][:]),
                    op0=AX.add, op1=AX.mult,
                    accum_out=stats_t[k][:, C_INT:C_INT + 1])
                nc.vector.scalar_tensor_tensor(
                    _fl(j2[:]), _fl(w5_t[k][:]), 1.0, _fl(c2_t[k][:]),
                    op0=AX.add, op1=AX.mult,
                    accum_out=stats_t[k][:, C_UNI:C_UNI + 1])

            def qfm(k):
                jq = jpool.tile([P, HB, W], BF16, tag="junk")
                nc.vector._custom_dve(
                    ATH_QFM, out=_fl(jq[:]), in0=_fl(c2_t[k][:]),
                    in1=_fl(vmaps[k][:]), s0=s0d0s[k][:, 1:2], s1=2.0 * EPS,
                    accum_out=stats_t[k][:, C_SG2:C_SG2 + 1])

            ybfs = {}

            def box1(k):
                # stage 1: Y = B @ t_norm (weight-major, 3 LDW) + evac + T
                yps = pspool.tile([P, HB, W], F32, tag="ps")
                for hb in range(HB):
                    nc.tensor.matmul(yps[:, hb, :], bD[:], tnb[:, HB * k + hb, :],
                                     start=True, stop=False)
                for hb in (1, 2, 3):
                    nc.tensor.matmul(yps[:, hb, :], bUp[:],
                                     tnb[:, HB * k + hb - 1, :],
                                     start=False, stop=(hb == 3))
                for hb in (0, 1, 2):
                    nc.tensor.matmul(yps[:, hb, :], bDn[:],
                                     tnb[:, HB * k + hb + 1, :],
                                     start=False, stop=True)
                ybf = ypool.tile([P, HB, W], BF16, tag="ybf")
                nc.scalar.copy(_fl(ybf[:]), _fl(yps[:]))
                ytb = ypool.tile([P, 16, P], BF16, tag="ytb")
                nc.sync.dma_start_transpose(ytb[:], ybf[:])
                ybfs[k] = ytb

            def box2(k):
                # stage 2 + fold of -961 tT into the same PSUM
                ytb = ybfs[k]
                btps = pspool.tile([P, HB, W], F32, tag="ps")
                ytb_r = ytb[:].rearrange("p (h c) k -> p c h k", c=4)
                for wb in range(HB):
                    nc.tensor.matmul(btps[:, wb, :], bDb[:], ytb_r[:, wb],
                                     start=True, stop=False)
                for wb in (1, 2, 3):
                    nc.tensor.matmul(btps[:, wb, :], bUpb[:], ytb_r[:, wb - 1],
                                     start=False, stop=False)
                for wb in (0, 1, 2):
                    nc.tensor.matmul(btps[:, wb, :], bDnb[:], ytb_r[:, wb + 1],
                                     start=False, stop=False)
                for wb in range(HB):
                    nc.tensor.matmul(btps[:, wb, :], nI[:],
                                     tbf[:, HB * k + wb, :],
                                     start=False, stop=True)
                # W5T = (5/961)|btps|, accum -> Sw5
                w5 = wpool.tile([P, HB, W], BF16, tag="w5")
                w5_t.append(w5)
                nc.scalar.activation(_fl(w5[:]), _fl(btps[:]),
                                     mybir.ActivationFunctionType.Abs,
                                     scale=5.0 / 961.0,
                                     accum_out=stats_t[k][:, C_SW5:C_SW5 + 1])

            # ------------- main software-pipelined loop -------------
            for i in range(IMG):
                sl = slice(HB * i, HB * i + HB)
                x_i, t_i, p_i = xbf[:, sl, :], tbf[:, sl, :], pbf[:, sl, :]

                # DVE maps
                pt = ppool.tile([P, HB, W], BF16, tag=f"pt{i}")
                c2 = ppool.tile([P, HB, W], BF16, tag=f"c2{i}")
                pt_t.append(pt)
                c2_t.append(c2)
                j0 = jpool.tile([P, HB, W], BF16, tag="junk")
                nc.vector.scalar_tensor_tensor(
                    _fl(j0[:]), _fl(t_i), 0.0, _fl(x_i),
                    op0=AX.bypass, op1=AX.mult,
                    accum_out=stats_t[i][:, C_STX:C_STX + 1])
                nc.vector.tensor_tensor(_fl(pt[:]), _fl(p_i), _fl(t_i),
                                        op=AX.mult)
                nc.vector.scalar_tensor_tensor(
                    _fl(c2[:]), _fl(p_i), 0.0, _fl(t_i),
                    op0=AX.bypass, op1=AX.add,
                    accum_out=stats_t[i][:, C_SC2:C_SC2 + 1])
                # means all-reduce as soon as Sp/Sc2 exist
                msum = mpool.tile([P, 2], F32, tag="msum")
                msums.append(msum)
                nc.gpsimd.partition_all_reduce(
                    msum[:], stats_t[i][:, C_SP:C_SP + 2], channels=P,
                    reduce_op=bass_isa.ReduceOp.add)
                # E-measure for previous image (its msum is long ready)
                if i >= 1:
                    emeasure(i - 1)

                if i == 0:
                    box1(0)
                if i + 1 < IMG:
                    box1(i + 1)   # fill PE's transpose-wait bubble
                box2(i)
                if i >= 1:
                    qfm(i - 1)
                    scsums(i - 1)

            # tail: E-measure + qfm + weighted sums of the last image
            emeasure(IMG - 1)
            qfm(IMG - 1)
            scsums(IMG - 1)
            # global sum(ln p) -> stats of image 3 (last reduced anyway);
            # dead last on the Scalar queue so it never stalls the pipeline.
            # (dump the map over xbf: x is fully consumed by now)
            nc.scalar.activation(_fl(xbf[:]), _fl(pbf[:]),
                                 mybir.ActivationFunctionType.Ln,
                                 accum_out=stats_t[IMG - 1][:, C_SLNP:C_SLNP + 1])

            # final partition reductions + output
            for i in range(IMG):
                nc.gpsimd.partition_all_reduce(
                    arout[:, SCOLS * i:SCOLS * (i + 1)], stats_t[i][:],
                    channels=P, reduce_op=bass_isa.ReduceOp.add)
            nc.sync.dma_start(outv, arout[0:1, :])

    nc.compile()
    return nc


_NC_CACHE = {}


def get_nc():
    if "nc" not in _NC_CACHE:
        _NC_CACHE["nc"] = build_nc()
    return _NC_CACHE["nc"]


# ------------------------------------------------------------- host side
def epilogue(parts):
    """parts: [8] arrays of [1, SCOLS*IMG] per-core stats -> scalar loss."""
    rows = np.concatenate([p.reshape(IMG, SCOLS) for p in parts], 0).astype(np.float64)
    sp = rows[:, C_SP]
    sc2 = rows[:, C_SC2]
    sw5 = rows[:, C_SW5]
    inter = rows[:, C_INT]
    union = rows[:, C_UNI]
    sg2 = rows[:, C_SG2]
    stx = rows[:, C_STX]
    slnp = rows[:, C_SLNP]

    # C_SLNP holds the per-core global sum(ln p) in image-3 rows only.
    bce = (-slnp[IMG - 1::IMG].sum() - stx.sum()) / (32 * NPIX)
    w_sum = NPIX + sw5
    w_bce = (w_sum * bce + EPS) / (w_sum + EPS)
    w_iou = 1.0 - (inter + 1.0 + EPS) / (union - inter + 1.0 + EPS)
    eloss = 1.0 - sg2 / (4.0 * NPIX)
    return np.float32((w_bce + w_iou + eloss).mean())


def make_in_maps(y_pred, y_target):
    consts = band_consts()
    bf = ml_dtypes.bfloat16
    x = np.asarray(y_pred, np.float32).reshape(32, 512, 512)
    t = np.asarray(y_target, np.float32).reshape(32, 512, 512)
    f8 = ml_dtypes.float8_e4m3fn
    xT = np.ascontiguousarray(x.transpose(0, 2, 1)).astype(f8)
    tT = np.ascontiguousarray(t.transpose(0, 2, 1)).astype(bf)
    tn = t.astype(f8)
    in_maps = []
    for c in range(N_CORES):
        s = slice(IMG * c, IMG * c + IMG)
        m = {
            "xT": np.ascontiguousarray(xT[s]),
            "tT": np.ascontiguousarray(tT[s]),
            "tn": np.ascontiguousarray(tn[s]),
        }
        m.update(consts)
        in_maps.append(m)
    return in_maps


def kernel(y_pred: np.ndarray, y_target: np.ndarray) -> np.ndarray:
    nc = get_nc()
    res = run_bass_kernel_spmd(nc, make_in_maps(y_pred, y_target),
                               core_ids=list(range(N_CORES)))
    parts = [res.results[c]["out"] for c in range(N_CORES)]
    return epilogue(parts)


# revision 24
# speedup vs baseline: 1.1859x; 1.1859x over previous
"""AdptWeightBCEDiceLoss on 8 TRN2 NeuronCores — pure data parallel.

Full inputs y_pred/y_target [32,1,512,512] f32 are cast to bf16 on host
and sharded 4 images per core.  All elementwise work runs in TRANSPOSED
image space (host also ships x^T and t^T); only box-filter stage 1
consumes t in normal orientation.  Each core emits per-image partial
sums; the host combines 8x12x4 partials into the final scalar (the
"all-reduce" of the sharding hint).

Per image (N = 512*512, e = 1e-8, s = 5/961):
  p     = sigmoid(xT)                      ScalarE, accum -> Sp
  Y     = B_ones @ t   (col box sum)       TensorE -> PSUM, evac bf16
  btps  = B_ones @ Y^T - 961 tT            TensorE (fold into one PSUM)
  W5T   = s|btps|                          ScalarE Abs, accum -> Sw5
  c2T   = pT+tT          (accum Sc2)       DVE STT
  ptT   = pT*tT                            DVE TT (2x mode)
  inter = sum(ptT*(1+W5T))                 DVE STT (scalar port adds 1)
  union = sum(c2T*(1+W5T))                 DVE STT
  Stx   = sum(tT*xT)                       DVE STT
  q     = (pT-mp)^2 + (tT-mt)^2            DVE custom
  v     = (q+e)^-1/2                       ScalarE Abs_reciprocal_sqrt
  Sg2   = sum((((c2T-S0)^2+2e) * v^2)^2)   DVE custom w/ accum (= sum G^2)
  Slnp  = sum(ln pT) (global)              ScalarE Ln
"""

import numpy as np
import ml_dtypes

import concourse.bass as bass
import concourse.mybir as mybir
import concourse.tile as tile
import concourse.bacc as bacc
from concourse.bass_utils import run_bass_kernel_spmd

# ---------------------------------------------------------------- constants
N_CORES = 8
IMG = 4           # images per core
HB = 4            # 128-row blocks per image
W = 512
P = 128
NPIX = 512 * 512
EPS = 1e-8
SCOLS = 12        # stats columns per image
F32 = mybir.dt.float32
BF16 = mybir.dt.bfloat16
FP8 = mybir.dt.float8e4
AX = mybir.AluOpType

# stats columns (per image)
C_SP, C_SC2, C_SW5, C_INT, C_UNI, C_SG2, C_STX, C_SLNP = range(8)

# ------------------------------------------------------- custom DVE ops
from concourse import dve_ops as _dvo
from concourse.dve_spec import Spec, Src0, Src1, C0, C1, C2, sq, lower, _has_src1, AluOp
from concourse.dve_uop import DveOpSpec


def _register(name, spec, subdim=False):
    for op in _dvo.OPS:
        if op.name == name:
            return op
    row = _dvo._CUSTOM_DVE_ROW_BASE + len(_dvo.OPS)
    assert row < 0x20
    shas = {}
    for ver in ("v3",):
        tmp = DveOpSpec(name=name, opcode=row, uops=lower(spec, ver=ver),
                        rd1_en=_has_src1(spec))
        shas[ver] = tmp.sha(ver)
    op = _dvo.DveOp(name, spec, subdim, shas)
    _dvo.OPS.append(op)
    _dvo._SUB_OPCODE_FOR_NAME[name] = row
    _dvo.CUSTOM_DVE_SPECS[name] = spec
    return op


# q = (p - mp)^2 + (t - mt)^2
ATH_Q = _register(
    "ATH_Q2",
    Spec(
        body=sq(Src0 - C0) + sq(Src1 - C1),
        reference=lambda in0, in1, s0, s1, imm2: (
            (in0.astype(np.float32) - s0) ** 2
            + (in1.astype(np.float32) - s1) ** 2
        ).astype(np.float32),
    ),
)

# G^2 = (((c2 - S0)^2 + 2e) * v^2)^2 with v = rsqrt(q+e); accum -> sum G^2
ATH_QFM = _register(
    "ATH_QFM",
    Spec(
        body=sq((sq(Src0 - C0) + C1) * sq(Src1)),
        accum=AluOp.ADD,
        reference=lambda in0, in1, s0, s1, imm2: (
            (((in0.astype(np.float32) - s0) ** 2 + s1)
             * in1.astype(np.float32) ** 2) ** 2
        ).astype(np.float32),
    ),
)


def band_consts():
    idx = np.arange(256)
    B = (np.abs(idx[:, None] - idx[None, :]) <= 15).astype(np.float32)
    D = B[0:128, 0:128]      # same-block band (ones)
    UP = B[0:128, 128:256]   # input block j = out block - 1
    DN = B[128:256, 0:128]   # input block j = out block + 1
    I = np.eye(128, dtype=np.float32)
    bf = ml_dtypes.bfloat16
    f8 = ml_dtypes.float8_e4m3fn
    return {
        "bandD": D.astype(f8), "bandUp": UP.astype(f8), "bandDn": DN.astype(f8),
        "bandDb": D.astype(bf), "bandUpb": UP.astype(bf), "bandDnb": DN.astype(bf),
        "negI961": (-961.0 * I).astype(bf),
    }


def _fl(ap):
    if len(ap.shape) == 3:
        return ap.rearrange("p a b -> p (a b)")
    if len(ap.shape) == 4:
        return ap.rearrange("p a b c -> p (a b c)")
    return ap


# ------------------------------------------------------------- builder
import os
SKIP = set(os.environ.get("ATH_SKIP", "").split(","))


def build_nc():
    nc = bacc.Bacc("TRN2", target_bir_lowering=False, debug=False,
                   num_devices=N_CORES)
    # xT/tT: transposed images [i, w, h]; tn: normal [i, h, w]
    xTd = nc.dram_tensor("xT", [IMG, 512, 512], FP8, kind="ExternalInput").ap()
    tTd = nc.dram_tensor("tT", [IMG, 512, 512], BF16, kind="ExternalInput").ap()
    tnd = nc.dram_tensor("tn", [IMG, 512, 512], FP8, kind="ExternalInput").ap()
    cD = nc.dram_tensor("bandD", [P, P], FP8, kind="ExternalInput").ap()
    cUp = nc.dram_tensor("bandUp", [P, P], FP8, kind="ExternalInput").ap()
    cDn = nc.dram_tensor("bandDn", [P, P], FP8, kind="ExternalInput").ap()
    cNI = nc.dram_tensor("negI961", [P, P], BF16, kind="ExternalInput").ap()
    cDb = nc.dram_tensor("bandDb", [P, P], BF16, kind="ExternalInput").ap()
    cUpb = nc.dram_tensor("bandUpb", [P, P], BF16, kind="ExternalInput").ap()
    cDnb = nc.dram_tensor("bandDnb", [P, P], BF16, kind="ExternalInput").ap()
    outv = nc.dram_tensor("out", [1, SCOLS * IMG], F32, kind="ExternalOutput").ap()

    xTv = xTd.rearrange("i (b p) h -> p (i b) h", p=P)
    tTv = tTd.rearrange("i (b p) h -> p (i b) h", p=P)
    tnv = tnd.rearrange("i (b p) w -> p (i b) w", p=P)

    with tile.TileContext(nc) as tc:
        import contextlib
        ctx = contextlib.ExitStack()
        with ctx:
            cpool = ctx.enter_context(tc.tile_pool(name="consts", bufs=1))
            dpool = ctx.enter_context(tc.tile_pool(name="data", bufs=1))
            ppool = ctx.enter_context(tc.tile_pool(name="ptc2", bufs=1))
            wpool = ctx.enter_context(tc.tile_pool(name="w5", bufs=3))
            ypool = ctx.enter_context(tc.tile_pool(name="ybuf", bufs=2))
            jpool = ctx.enter_context(tc.tile_pool(name="junk", bufs=4))
            qpool = ctx.enter_context(tc.tile_pool(name="qv", bufs=2))
            mpool = ctx.enter_context(tc.tile_pool(name="means", bufs=4))
            pspool = ctx.enter_context(tc.tile_pool(name="ps", bufs=2, space="PSUM"))

            bD = cpool.tile([P, P], FP8, tag="bD")
            bUp = cpool.tile([P, P], FP8, tag="bUp")
            bDn = cpool.tile([P, P], FP8, tag="bDn")
            nI = cpool.tile([P, P], BF16, tag="nI")
            bDb = cpool.tile([P, P], BF16, tag="bDb")
            bUpb = cpool.tile([P, P], BF16, tag="bUpb")
            bDnb = cpool.tile([P, P], BF16, tag="bDnb")
            ebias = cpool.tile([P, 1], F32, tag="ebias")
            nc.gpsimd.memset(ebias[:], EPS)

            xbf = dpool.tile([P, IMG * HB, W], FP8, tag="xbf")   # xT
            tbf = dpool.tile([P, IMG * HB, W], BF16, tag="tbf")   # tT
            tnb = dpool.tile([P, IMG * HB, W], FP8, tag="tnb")    # t normal
            pbf = dpool.tile([P, IMG * HB, W], BF16, tag="pbf")   # pT
            stats_t = []
            for j in range(IMG):
                st_j = dpool.tile([P, SCOLS], F32, tag=f"stats{j}")
                stats_t.append(st_j)
            arout = dpool.tile([P, SCOLS * IMG], F32, tag="arout")
            pt_t, c2_t, w5_t, msums, s0d0s, vmaps = [], [], [], [], [], []

            # consts via SWDGE (tiny, land early); inputs as per-image trios
            # on the sync queue so image 0 is complete ASAP.
            nc.gpsimd.dma_start(out=bD[:], in_=cD)
            nc.gpsimd.dma_start(out=bUp[:], in_=cUp)
            nc.gpsimd.dma_start(out=bDn[:], in_=cDn)
            nc.gpsimd.dma_start(out=nI[:], in_=cNI)
            nc.gpsimd.dma_start(out=bDb[:], in_=cDb)
            nc.gpsimd.dma_start(out=bUpb[:], in_=cUpb)
            nc.gpsimd.dma_start(out=bDnb[:], in_=cDnb)
            for i in range(IMG):
                sl = slice(HB * i, HB * i + HB)
                nc.sync.dma_start(tnb[:, sl, :], tnv[:, sl, :])
                nc.sync.dma_start(xbf[:, sl, :], xTv[:, sl, :])
                nc.sync.dma_start(tbf[:, sl, :], tTv[:, sl, :])

            import concourse.bass_isa as bass_isa

            # all sigmoids first: Scalar queue never returns to this table
            for i in range(IMG):
                sl = slice(HB * i, HB * i + HB)
                nc.scalar.activation(_fl(pbf[:, sl, :]), _fl(xbf[:, sl, :]),
                                     mybir.ActivationFunctionType.Sigmoid,
                                     accum_out=stats_t[i][:, C_SP:C_SP + 1])

            def emeasure(k):
                """E-measure chain for image k (maps built in iter k)."""
                sl = slice(HB * k, HB * k + HB)
                msum = msums[k]
                s0d0 = mpool.tile([P, 3], F32, tag="s0d0")  # [mp, mt, S0]
                # [mp, S0] in one op, then mt = S0 - mp
                nc.vector.tensor_single_scalar(s0d0[:, 0:2], msum[:, 0:2],
                                               1.0 / NPIX, AX.mult)
                nc.vector.scalar_tensor_tensor(
                    s0d0[:, 2:3], s0d0[:, 0:1], -1.0, s0d0[:, 1:2],
                    op0=AX.mult, op1=AX.add)  # mt = S0 - mp
                q = qpool.tile([P, HB, W], BF16, tag="q")
                nc.vector._custom_dve(ATH_Q, out=_fl(q[:]),
                                      in0=_fl(pbf[:, sl, :]),
                                      in1=_fl(tbf[:, sl, :]),
                                      s0=s0d0[:, 0:1], s1=s0d0[:, 2:3])
                v = qpool.tile([P, HB, W], BF16, tag="v")
                nc.scalar.activation(
                    _fl(v[:]), _fl(q[:]),
                    mybir.ActivationFunctionType.Abs_reciprocal_sqrt,
                    bias=ebias[:])
                s0d0s.append(s0d0)
                vmaps.append(v)

            def scsums(k):
                # inter = sum(pt*(1+W5)), union = sum(c2*(1+W5)); the +1 is
                # folded into the STT scalar port
                j1 = jpool.tile([P, HB, W], BF16, tag="junk")
                j2 = jpool.tile([P, HB, W], BF16, tag="junk")
                nc.vector.scalar_tensor_tensor(
                    _fl(j1[:]), _fl(w5_t[k][:]), 1.0, _fl(pt_t[k][:]),
                    op0=AX.add, op1=AX.mult,
                    accum_out=stats_t[k][:, C_INT:C_INT + 1])
                nc.vector.scalar_tensor_tensor(
                    _fl(j2[:]), _fl(w5_t[k][:]), 1.0, _fl(c2_t[k][:]),
                    op0=AX.add, op1=AX.mult,
                    accum_out=stats_t[k][:, C_UNI:C_UNI + 1])

            def qfm(k):
                jq = jpool.tile([P, HB, W], BF16, tag="junk")
                nc.vector._custom_dve(
                    ATH_QFM, out=_fl(jq[:]), in0=_fl(c2_t[k][:]),
                    in1=_fl(vmaps[k][:]), s0=s0d0s[k][:, 1:2], s1=2.0 * EPS,
                    accum_out=stats_t[k][:, C_SG2:C_SG2 + 1])

            ybfs = {}

            def box1(k):
                # stage 1: Y = B @ t_norm (weight-major, 3 LDW) + evac + T
                yps = pspool.tile([P, HB, W], F32, tag="ps")
                for hb in range(HB):
                    nc.tensor.matmul(yps[:, hb, :], bD[:], tnb[:, HB * k + hb, :],
                                     start=True, stop=False)
                for hb in (1, 2, 3):
                    nc.tensor.matmul(yps[:, hb, :], bUp[:],
                                     tnb[:, HB * k + hb - 1, :],
                                     start=False, stop=(hb == 3))
                for hb in (0, 1, 2):
                    nc.tensor.matmul(yps[:, hb, :], bDn[:],
                                     tnb[:, HB * k + hb + 1, :],
                                     start=False, stop=True)
                ybf = ypool.tile([P, HB, W], BF16, tag="ybf")
                nc.scalar.copy(_fl(ybf[:]), _fl(yps[:]))
                ytb = ypool.tile([P, 16, P], BF16, tag="ytb")
                nc.sync.dma_start_transpose(ytb[:], ybf[:])
                ybfs[k] = ytb

            def box2(k):
                # stage 2 + fold of -961 tT into the same PSUM
                ytb = ybfs[k]
                btps = pspool.tile([P, HB, W], F32, tag="ps")
                ytb_r = ytb[:].rearrange("p (h c) k -> p c h k", c=4)
                for wb in range(HB):
                    nc.tensor.matmul(btps[:, wb, :], bDb[:], ytb_r[:, wb],
                                     start=True, stop=False)
                for wb in (1, 2, 3):
                    nc.tensor.matmul(btps[:, wb, :], bUpb[:], ytb_r[:, wb - 1],
                                     start=False, stop=False)
                for wb in (0, 1, 2):
                    nc.tensor.matmul(btps[:, wb, :], bDnb[:], ytb_r[:, wb + 1],
                                     start=False, stop=False)
                for wb in range(HB):
                    nc.tensor.matmul(btps[:, wb, :], nI[:],
                                     tbf[:, HB * k + wb, :],
                                     start=False, stop=True)
                # W5T = (5/961)|btps|, accum -> Sw5
                w5 = wpool.tile([P, HB, W], BF16, tag="w5")
                w5_t.append(w5)
                nc.scalar.activation(_fl(w5[:]), _fl(btps[:]),
                                     mybir.ActivationFunctionType.Abs,
                                     scale=5.0 / 961.0,
                                     accum_out=stats_t[k][:, C_SW5:C_SW5 + 1])

            # ------------- main software-pipelined loop -------------
            for i in range(IMG):
                sl = slice(HB * i, HB * i + HB)
                x_i, t_i, p_i = xbf[:, sl, :], tbf[:, sl, :], pbf[:, sl, :]

                # DVE maps
                pt = ppool.tile([P, HB, W], BF16, tag=f"pt{i}")
                c2 = ppool.tile([P, HB, W], BF16, tag=f"c2{i}")
                pt_t.append(pt)
                c2_t.append(c2)
                j0 = jpool.tile([P, HB, W], BF16, tag="junk")
                nc.vector.scalar_tensor_tensor(
                    _fl(j0[:]), _fl(t_i), 0.0, _fl(x_i),
                    op0=AX.bypass, op1=AX.mult,
                    accum_out=stats_t[i][:, C_STX:C_STX + 1])
                nc.vector.tensor_tensor(_fl(pt[:]), _fl(p_i), _fl(t_i),
                                        op=AX.mult)
                nc.vector.scalar_tensor_tensor(
                    _fl(c2[:]), _fl(p_i), 0.0, _fl(t_i),
                    op0=AX.bypass, op1=AX.add,
                    accum_out=stats_t[i][:, C_SC2:C_SC2 + 1])
                # means all-reduce as soon as Sp/Sc2 exist
                msum = mpool.tile([P, 2], F32, tag="msum")
                msums.append(msum)
                nc.gpsimd.partition_all_reduce(
                    msum[:], stats_t[i][:, C_SP:C_SP + 2], channels=P,
                    reduce_op=bass_isa.ReduceOp.add)
                # E-measure for previous image (its msum is long ready)
                if i >= 1:
                    emeasure(i - 1)

                if i == 0:
                    box1(0)
                if i + 1 < IMG:
                    box1(i + 1)   # fill PE's transpose-wait bubble
                box2(i)
                if i >= 1:
                    qfm(i - 1)
                    scsums(i - 1)

            # tail: E-measure + qfm + weighted sums of the last image
            emeasure(IMG - 1)
            qfm(IMG - 1)
            scsums(IMG - 1)
            # global sum(ln p) -> stats of image 3 (last reduced anyway);
            # dead last on the Scalar queue so it never stalls the pipeline.
            # (dump the map over xbf: x is fully consumed by now)
            nc.scalar.activation(_fl(xbf[:]), _fl(pbf[:]),
                                 mybir.ActivationFunctionType.Ln,
                                 accum_out=stats_t[IMG - 1][:, C_SLNP:C_SLNP + 1])

            # final partition reductions + output
            for i in range(IMG):
                nc.gpsimd.partition_all_reduce(
                    arout[:, SCOLS * i:SCOLS * (i + 1)], stats_t[i][:],
                    channels=P, reduce_op=bass_isa.ReduceOp.add)
            nc.sync.dma_start(outv, arout[0:1, :])

    nc.compile()
    return nc


_NC_CACHE = {}


def get_nc():
    if "nc" not in _NC_CACHE:
        _NC_CACHE["nc"] = build_nc()
    return _NC_CACHE["nc"]


# ------------------------------------------------------------- host side
def epilogue(parts):
    """parts: [8] arrays of [1, SCOLS*IMG] per-core stats -> scalar loss."""
    rows = np.concatenate([p.reshape(IMG, SCOLS) for p in parts], 0).astype(np.float64)
    sp = rows[:, C_SP]
    sc2 = rows[:, C_SC2]
    sw5 = rows[:, C_SW5]
    inter = rows[:, C_INT]
    union = rows[:, C_UNI]
    sg2 = rows[:, C_SG2]
    stx = rows[:, C_STX]
    slnp = rows[:, C_SLNP]

    # C_SLNP holds the per-core global sum(ln p) in image-3 rows only.
    bce = (-slnp[IMG - 1::IMG].sum() - stx.sum()) / (32 * NPIX)
    w_sum = NPIX + sw5
    w_bce = (w_sum * bce + EPS) / (w_sum + EPS)
    w_iou = 1.0 - (inter + 1.0 + EPS) / (union - inter + 1.0 + EPS)
    eloss = 1.0 - sg2 / (4.0 * NPIX)
    return np.float32((w_bce + w_iou + eloss).mean())


def make_in_maps(y_pred, y_target):
    consts = band_consts()
    bf = ml_dtypes.bfloat16
    x = np.asarray(y_pred, np.float32).reshape(32, 512, 512)
    t = np.asarray(y_target, np.float32).reshape(32, 512, 512)
    f8 = ml_dtypes.float8_e4m3fn
    xT = np.ascontiguousarray(x.transpose(0, 2, 1)).astype(f8)
    tT = np.ascontiguousarray(t.transpose(0, 2, 1)).astype(bf)
    tn = t.astype(f8)
    in_maps = []
    for c in range(N_CORES):
        s = slice(IMG * c, IMG * c + IMG)
        m = {
            "xT": np.ascontiguousarray(xT[s]),
            "tT": np.ascontiguousarray(tT[s]),
            "tn": np.ascontiguousarray(tn[s]),
        }
        m.update(consts)
        in_maps.append(m)
    return in_maps


def kernel(y_pred: np.ndarray, y_target: np.ndarray) -> np.ndarray:
    nc = get_nc()
    res = run_bass_kernel_spmd(nc, make_in_maps(y_pred, y_target),
                               core_ids=list(range(N_CORES)))
    parts = [res.results[c]["out"] for c in range(N_CORES)]
    return epilogue(parts)
